# revision 3
# baseline (speedup 1.0000x reference)
import sys

sys.path.insert(0, "/opt/trn_rl_repo")

import numpy as np
import bass_rust
import concourse.bass as bass
import concourse.mybir as mybir
import concourse.tile as tile
from concourse.bass_utils import run_bass_kernel_spmd

import concourse.dve_ops as dve_ops
from concourse.dve_ops import DveOp
from concourse.dve_spec import Spec, Src0, C0, C1, C2, One, maxx, relu, sq, lower, _has_src1
from concourse.dve_uop import DveOpSpec

B, H, NCORES = 4096, 1024, 8
NB = B // NCORES          # 512 batch rows per core
FB = H // 128             # 8 feature blocks of 128
NPL = 8                   # spline planes per feature (phase A)
NPL_B = 7                 # phase B: |o|<1 => partition of unity folds plane 7
KT_A = 2 * FB * (1 + NPL)  # 144 k-tiles: kan_x + kan_h
KT_B = FB + FB * NPL_B     # 64 k-tiles: hh_w + kan_hz (silu folded into planes)
FP32 = mybir.dt.float32
BF16 = mybir.dt.bfloat16
BF16_NP = mybir.dt.np(BF16)
AF = mybir.ActivationFunctionType


def _register_dve(name, spec):
    if name in dve_ops._SUB_OPCODE_FOR_NAME:
        return next(op for op in dve_ops.OPS if op.name == name)
    row = dve_ops._CUSTOM_DVE_ROW_BASE + len(dve_ops.OPS)
    assert row < 0x20
    dve_ops._SUB_OPCODE_FOR_NAME[name] = row
    shas = {v: DveOpSpec(name=name, opcode=row, uops=lower(spec, ver=v),
                         rd1_en=_has_src1(spec)).sha(v) for v in ("v3", "v4")}
    op = DveOp(name, spec, subdim=False, uops_sha=shas)
    dve_ops.OPS.append(op)
    dve_ops.CUSTOM_DVE_SPECS[name] = op.spec
    return op


# p_c = relu(2 - |2.5*x - (c - 3.5)|), then plane = p^3 - 4*relu(p-1)^3 = 6*B3(u-c)
_m = Src0 * C2
TENT = _register_dve("KAN_TENT", Spec(
    body=relu(C1 - maxx(_m - C0, C0 - _m)),
    reference=lambda in0, in1, s0, s1, imm2:
        np.maximum(s1 - np.abs(in0.astype(np.float32) * imm2 - s0), 0.0)))
_q = relu(Src0 - One)
CUBE = _register_dve("KAN_CUBE", Spec(
    body=sq(Src0) * Src0 - sq(_q) * _q * C2,
    reference=lambda in0, in1, s0, s1, imm2:
        in0.astype(np.float32) ** 3
        - imm2 * np.maximum(in0.astype(np.float32) - 1.0, 0.0) ** 3))


def _beta_silu():
    # coefficients of silu in the 8-fn cubic B-spline basis on [-1,1]
    # (max residual ~1.9e-5); lets phase B drop its silu k-tiles entirely
    x = np.linspace(-1.0, 1.0, 4001)
    c = np.arange(NPL)[None, :]
    p = np.maximum(2.0 - np.abs(2.5 * x[:, None] - (c - 3.5)), 0.0)
    A = (p ** 3 - 4.0 * np.maximum(p - 1.0, 0.0) ** 3) / 6.0
    silu = x / (1.0 + np.exp(-x))
    return np.linalg.lstsq(A, silu, rcond=None)[0].astype(np.float32)


BETA_SILU = _beta_silu()


def _build(reps=1, mode="full"):
    # mode: "full" | "pe" (no weight streaming, one resident w tile)
    #       | "dma" (weight streaming only, no compute)
    wdt = BF16
    nc = bass.Bass(target_bir_lowering=False)
    xs_d = nc.dram_tensor("xs", [128, FB * NB], FP32, kind="ExternalInput")
    zs_d = nc.dram_tensor("zs", [128, FB * NB], FP32, kind="ExternalInput")
    wa_d = nc.dram_tensor("wa", [KT_A * 128, H], wdt, kind="ExternalInput")
    wb_d = nc.dram_tensor("wb", [KT_B * 128, H], wdt, kind="ExternalInput")
    hb_d = nc.dram_tensor("hb", [128, FB], FP32, kind="ExternalInput")
    o_d = nc.dram_tensor("o", [128, FB * NB], FP32, kind="ExternalOutput")
    zo_d = nc.dram_tensor("zo", [128, FB * NB], BF16, kind="ExternalOutput")

    with tile.TileContext(nc) as tc:
        with tc.tile_pool(name="sbuf", bufs=1) as pool, \
             tc.tile_pool(name="wp", bufs=16) as wpool, \
             tc.tile_pool(name="cp", bufs=2) as cpool, \
             tc.tile_pool(name="dp", bufs=2) as dpool, \
             tc.tile_pool(name="psum", bufs=1, space="PSUM") as psum:
            zb = pool.tile([128, FB * NB], BF16)   # bf16 copy of z for hh matmul
            oT = pool.tile([128, FB * NB], FP32)
            zt = pool.tile([128, FB * NB], BF16)
            hb = pool.tile([128, FB], FP32)

            ps = [psum.tile([128, NB], FP32, name=f"ps{mt}") for mt in range(FB)]

            w_res = None
            if mode == "pe":
                w_res = pool.tile([128, H], wdt)
                nc.sync.dma_start(w_res[:], wa_d[0:128, :])

            def wtile(wd, kt):
                if mode == "pe":
                    return w_res
                w = wpool.tile([128, H], wdt, name="w")
                nc.sync.dma_start(w[:], wd[kt * 128:(kt + 1) * 128, :])
                return w

            def dve_rhs(src_fp32, npl, with_sil):
                # [silu +] npl spline planes; per-plane TENT->CUBE so plane c
                # is ready early (the PE consumes plane c only at k-tile c+1)
                rhs = []
                if with_sil:
                    sil = dpool.tile([128, NB], BF16, name="sil")
                    nc.scalar.activation(sil[:], src_fp32, AF.Silu)
                    rhs.append(sil[:])
                tent = dpool.tile([128, NPL * NB], FP32, name="tent")
                planes = dpool.tile([128, NPL * NB], BF16, name="planes")
                for c in range(npl):
                    nc.vector._custom_dve(TENT, out=tent[:, c * NB:(c + 1) * NB],
                                          in0=src_fp32, s0=float(c) - 3.5, s1=2.0,
                                          imm2=2.5)
                    nc.vector._custom_dve(CUBE, out=planes[:, c * NB:(c + 1) * NB],
                                          in0=tent[:, c * NB:(c + 1) * NB], imm2=4.0)
                    rhs.append(planes[:, c * NB:(c + 1) * NB])
                return rhs

            def group(src_fp32, wd, kt, npl, with_sil=True, is_start=False):
                # j-inner: one feature block -> (with_sil + npl) k-tiles
                rhs_list = dve_rhs(src_fp32, npl, with_sil)
                for j, rhs in enumerate(rhs_list):
                    w = wtile(wd, kt + j)
                    for mt in range(FB):
                        nc.tensor.matmul(ps[mt][:], w[:, mt * 128:(mt + 1) * 128],
                                         rhs, start=(is_start and kt + j == 0),
                                         stop=False)
                return kt + len(rhs_list)

            def group_mt_outer(src_fp32, wd, kt, npl, epilogue, with_sil=True):
                # last group of a phase: mt-outer so ps[mt] finishes early and
                # the per-mt epilogue (act + store) overlaps remaining matmuls
                rhs_list = dve_rhs(src_fp32, npl, with_sil)
                ws = [wtile(wd, kt + j) for j in range(len(rhs_list))]
                last = len(rhs_list) - 1
                for mt in range(FB):
                    for j, rhs in enumerate(rhs_list):
                        nc.tensor.matmul(ps[mt][:], ws[j][:, mt * 128:(mt + 1) * 128],
                                         rhs, start=False, stop=(j == last))
                    epilogue(mt)
                return kt + len(rhs_list)

            def dma_only():
                for kt in range(KT_A):
                    w = wpool.tile([128, H], wdt, name="w")
                    nc.sync.dma_start(w[:], wa_d[kt * 128:(kt + 1) * 128, :])
                for kt in range(KT_B):
                    w = wpool.tile([128, H], wdt, name="w")
                    nc.sync.dma_start(w[:], wb_d[kt * 128:(kt + 1) * 128, :])

            def ep_a(mt):
                # tanh into oT; store o in 1024-wide chunks (fewer DGE configs)
                sl = slice(mt * NB, (mt + 1) * NB)
                nc.scalar.activation(oT[:, sl], ps[mt][:], AF.Tanh)
                if mt % 2 == 1:
                    sl2 = slice((mt - 1) * NB, (mt + 1) * NB)
                    nc.scalar.dma_start(o_d[:, sl2], oT[:, sl2])

            def ep_b(mt):
                sl = slice(mt * NB, (mt + 1) * NB)
                nc.scalar.activation(zt[:, sl], ps[mt][:], AF.Identity,
                                     bias=hb[:, mt:mt + 1], scale=1.0)
                # 1024-wide chunks early; the last two on the idle SP queue so
                # the DGE config overlaps the act and the tail latency shrinks
                if mt in (1, 3, 5):
                    sl2 = slice((mt - 1) * NB, (mt + 1) * NB)
                    nc.scalar.dma_start(zo_d[:, sl2], zt[:, sl2])
                elif mt in (6, 7):
                    nc.sync.dma_start(zo_d[:, sl], zt[:, sl])

            for _rep in range(reps):
                if mode == "dma":
                    dma_only()
                    continue
                # phase A: s = kan_x(x) + kan_h(z); o = tanh(s)
                # Input chunks come from a 2-slot pool: chunk g+2's DMA gains
                # a WAR dep on chunk g's last reader, which paces the loads at
                # group rate in HARDWARE (no startup flood racing the weight
                # stream for HBM). Chunk 0 goes program-first on the SP queue
                # (ahead of the weight tiles on the same FIFO) and the Act
                # queue carries nothing but [table load, silus, epilogues], so
                # the first matmul can issue as soon as xs0+w0 land.
                kt = 0
                for src_d, is_z in ((xs_d, False), (zs_d, True)):
                    for fb in range(FB):
                        sl = slice(fb * NB, (fb + 1) * NB)
                        chunk = cpool.tile([128, NB], FP32, name="chunk")
                        eng = nc.sync if kt == 0 else nc.gpsimd
                        eng.dma_start(chunk[:], src_d[:, sl])
                        if kt == KT_A - 1 - NPL:
                            kt = group_mt_outer(chunk[:], wa_d, kt, NPL, ep_a)
                        else:
                            kt = group(chunk[:], wa_d, kt, NPL, is_start=True)
                        if is_z:
                            # cast on the idle Pool engine: keeps the Act
                            # engine on silu only (no act-table switches)
                            nc.gpsimd.tensor_copy(zb[:, sl], chunk[:])

                # phase B: z_out = z @ hh_w.T + hb + kan_hz(o)
                nc.gpsimd.dma_start(hb[:], hb_d[:])
                kt = 0
                for fb in range(FB):
                    w = wtile(wb_d, kt)
                    rhs = zb[:, fb * NB:(fb + 1) * NB]
                    for mt in range(FB):
                        nc.tensor.matmul(ps[mt][:], w[:, mt * 128:(mt + 1) * 128],
                                         rhs, start=(kt == 0), stop=False)
                    kt += 1
                for fb in range(FB):
                    sl = slice(fb * NB, (fb + 1) * NB)
                    if fb == FB - 1:
                        kt = group_mt_outer(oT[:, sl], wb_d, kt, NPL_B, ep_b,
                                            with_sil=False)
                    else:
                        kt = group(oT[:, sl], wb_d, kt, NPL_B, with_sil=False)

    bass_rust.generate_event_semaphores(nc)
    mybir.codegen_inst_isa_subclasses(nc)
    return nc


_NCS = {}


def _get_nc(reps=1, mode="full"):
    key = (reps, mode)
    if key not in _NCS:
        _NCS[key] = _build(reps, mode)
    return _NCS[key]


def _to_dev(a):  # [NB, H] -> [128, FB*NB], block fb holds features fb*128..+128
    return np.ascontiguousarray(
        a.reshape(NB, FB, 128).transpose(2, 1, 0).reshape(128, FB * NB))


def _from_dev(a):  # [128, FB*NB] -> [NB, H]
    return a.reshape(128, FB, NB).transpose(2, 1, 0).reshape(NB, H)


def _pack_kan(wa, kt, bw, sw, sc):
    w2 = (np.asarray(sw, np.float32) * np.asarray(sc, np.float32)[:, :, None]) / 6.0
    bw = np.asarray(bw, np.float32)
    for fb in range(FB):
        wa[kt * 128:(kt + 1) * 128] = bw[:, fb * 128:(fb + 1) * 128].T
        kt += 1
        for c in range(NPL):
            wa[kt * 128:(kt + 1) * 128] = w2[:, fb * 128:(fb + 1) * 128, c].T
            kt += 1
    return kt


def _pack_kan_bounded(wb, kt, bw, sw, sc):
    # input bounded in (-1,1): silu folds into the spline basis (BETA_SILU)
    # and partition of unity folds plane 7 into a bias. Returns (kt, bias[o]).
    sw = np.asarray(sw, np.float32)
    sc = np.asarray(sc, np.float32)
    bw = np.asarray(bw, np.float32)
    wc = sw * sc[:, :, None] + bw[:, :, None] * BETA_SILU[None, None, :]
    bias = wc[:, :, NPL - 1].sum(axis=1)
    w2 = (wc[:, :, :NPL_B] - wc[:, :, NPL - 1:NPL]) / 6.0
    for fb in range(FB):
        for c in range(NPL_B):
            wb[kt * 128:(kt + 1) * 128] = w2[:, fb * 128:(fb + 1) * 128, c].T
            kt += 1
    return kt, bias


def _make_in_maps(inputs):
    x = np.ascontiguousarray(np.asarray(inputs["x_t"], np.float32))
    z = np.ascontiguousarray(np.asarray(inputs["z_prev"], np.float32))

    wa = np.empty((KT_A * 128, H), np.float32)
    kt = _pack_kan(wa, 0, inputs["wx_base"], inputs["wx_spline"], inputs["wx_scaler"])
    kt = _pack_kan(wa, kt, inputs["wh_base"], inputs["wh_spline"], inputs["wh_scaler"])
    assert kt == KT_A

    wb = np.empty((KT_B * 128, H), np.float32)
    wb[:H] = np.asarray(inputs["hh_w"], np.float32).T
    kt, hz_bias = _pack_kan_bounded(wb, FB, inputs["hz_base"], inputs["hz_spline"],
                                    inputs["hz_scaler"])
    assert kt == KT_B
    wa = np.ascontiguousarray(wa.astype(BF16_NP))
    wb = np.ascontiguousarray(wb.astype(BF16_NP))

    hbias = np.asarray(inputs["hh_b"], np.float32) + hz_bias
    hb = np.ascontiguousarray(hbias.reshape(FB, 128).T)

    return [{"xs": _to_dev(x[d * NB:(d + 1) * NB]),
             "zs": _to_dev(z[d * NB:(d + 1) * NB]),
             "wa": wa, "wb": wb, "hb": hb} for d in range(NCORES)]


def _run(inputs, trace=False):
    nc = _get_nc()
    in_maps = _make_in_maps(inputs)
    res = run_bass_kernel_spmd(nc, in_maps, list(range(NCORES)), trace=trace)
    o = np.empty((B, H), np.float32)
    zt = np.empty((B, H), np.float32)
    for d in range(NCORES):
        o[d * NB:(d + 1) * NB] = _from_dev(res.results[d]["o"])
        zt[d * NB:(d + 1) * NB] = _from_dev(
            np.asarray(res.results[d]["zo"]).astype(np.float32))
    return (o, zt), res


def kernel(**inputs):
    return _run(inputs, trace=False)[0]



# revision 7
# speedup vs baseline: 1.2039x; 1.2039x over previous
import sys

sys.path.insert(0, "/opt/trn_rl_repo")

import numpy as np
import bass_rust
import concourse.bass as bass
import concourse.mybir as mybir
import concourse.tile as tile
from concourse.bass_utils import run_bass_kernel_spmd

import concourse.dve_ops as dve_ops
from concourse.dve_ops import DveOp
from concourse.dve_spec import Spec, Src0, C0, C1, C2, One, maxx, relu, sq, lower, _has_src1
from concourse.dve_uop import DveOpSpec

B, H, NCORES = 4096, 1024, 8
NB = B // NCORES          # 512 batch rows per core
FB = H // 128             # 8 feature blocks of 128
NPL = 8                   # spline planes per feature (phase A)
NPL_B = 7                 # phase B: |o|<1 => partition of unity folds plane 7
KT_A = 2 * FB * (1 + NPL)  # 144 k-tiles: kan_x + kan_h
KT_B = FB + FB * NPL_B     # 64 k-tiles: hh_w + kan_hz (silu folded into planes)
FP32 = mybir.dt.float32
BF16 = mybir.dt.bfloat16
BF16_NP = mybir.dt.np(BF16)
AF = mybir.ActivationFunctionType


def _register_dve(name, spec):
    if name in dve_ops._SUB_OPCODE_FOR_NAME:
        return next(op for op in dve_ops.OPS if op.name == name)
    row = dve_ops._CUSTOM_DVE_ROW_BASE + len(dve_ops.OPS)
    assert row < 0x20
    dve_ops._SUB_OPCODE_FOR_NAME[name] = row
    shas = {v: DveOpSpec(name=name, opcode=row, uops=lower(spec, ver=v),
                         rd1_en=_has_src1(spec)).sha(v) for v in ("v3", "v4")}
    op = DveOp(name, spec, subdim=False, uops_sha=shas)
    dve_ops.OPS.append(op)
    dve_ops.CUSTOM_DVE_SPECS[name] = op.spec
    return op


# p_c = relu(2 - |2.5*x - (c - 3.5)|), then plane = p^3 - 4*relu(p-1)^3 = 6*B3(u-c)
_m = Src0 * C2
TENT = _register_dve("KAN_TENT", Spec(
    body=relu(C1 - maxx(_m - C0, C0 - _m)),
    reference=lambda in0, in1, s0, s1, imm2:
        np.maximum(s1 - np.abs(in0.astype(np.float32) * imm2 - s0), 0.0)))
_q = relu(Src0 - One)
CUBE = _register_dve("KAN_CUBE", Spec(
    body=sq(Src0) * Src0 - sq(_q) * _q * C2,
    reference=lambda in0, in1, s0, s1, imm2:
        in0.astype(np.float32) ** 3
        - imm2 * np.maximum(in0.astype(np.float32) - 1.0, 0.0) ** 3))


def _beta_silu():
    # coefficients of silu in the 8-fn cubic B-spline basis on [-1,1]
    # (max residual ~1.9e-5); lets phase B drop its silu k-tiles entirely
    x = np.linspace(-1.0, 1.0, 4001)
    c = np.arange(NPL)[None, :]
    p = np.maximum(2.0 - np.abs(2.5 * x[:, None] - (c - 3.5)), 0.0)
    A = (p ** 3 - 4.0 * np.maximum(p - 1.0, 0.0) ** 3) / 6.0
    silu = x / (1.0 + np.exp(-x))
    return np.linalg.lstsq(A, silu, rcond=None)[0].astype(np.float32)


BETA_SILU = _beta_silu()


def _build(reps=1, mode="full"):
    # mode: "full" | "pe" (no weight streaming, one resident w tile)
    #       | "dma" (weight streaming only, no compute)
    wdt = BF16
    nc = bass.Bass(target_bir_lowering=False)
    xs_d = nc.dram_tensor("xs", [128, FB * NB], FP32, kind="ExternalInput")
    zs_d = nc.dram_tensor("zs", [128, FB * NB], FP32, kind="ExternalInput")
    wa_d = nc.dram_tensor("wa", [KT_A * 128, H], wdt, kind="ExternalInput")
    wb_d = nc.dram_tensor("wb", [KT_B * 128, H], wdt, kind="ExternalInput")
    hb_d = nc.dram_tensor("hb", [128, FB], FP32, kind="ExternalInput")
    o_d = nc.dram_tensor("o", [128, FB * NB], FP32, kind="ExternalOutput")
    zo_d = nc.dram_tensor("zo", [128, FB * NB], BF16, kind="ExternalOutput")

    with tile.TileContext(nc) as tc:
        with tc.tile_pool(name="sbuf", bufs=1) as pool, \
             tc.tile_pool(name="wp", bufs=16) as wpool, \
             tc.tile_pool(name="cp", bufs=3) as cpool, \
             tc.tile_pool(name="dp", bufs=2) as dpool, \
             tc.tile_pool(name="psum", bufs=1, space="PSUM") as psum:
            zb = pool.tile([128, FB * NB], BF16)   # bf16 copy of z for hh matmul
            oT = pool.tile([128, FB * NB], FP32)
            zt = pool.tile([128, FB * NB], BF16)
            hb = pool.tile([128, FB], FP32)
            warm = pool.tile([128, NB], BF16)      # zeros; HAM warm-up operand

            ps = [psum.tile([128, NB], FP32, name=f"ps{mt}") for mt in range(FB)]

            w_res = None
            if mode == "pe":
                w_res = pool.tile([128, H], wdt)
                nc.sync.dma_start(w_res[:], wa_d[0:128, :])

            def wtile(wd, kt):
                if mode == "pe":
                    return w_res
                w = wpool.tile([128, H], wdt, name="w")
                nc.sync.dma_start(w[:], wd[kt * 128:(kt + 1) * 128, :])
                return w

            def dve_rhs(src_fp32, npl, with_sil):
                # [silu +] npl spline planes; per-plane TENT->CUBE so plane c
                # is ready early (the PE consumes plane c only at k-tile c+1)
                rhs = []
                if with_sil:
                    sil = dpool.tile([128, NB], BF16, name="sil")
                    nc.scalar.activation(sil[:], src_fp32, AF.Silu)
                    rhs.append(sil[:])
                tent = dpool.tile([128, NPL * NB], FP32, name="tent")
                planes = dpool.tile([128, NPL * NB], BF16, name="planes")
                for c in range(npl):
                    nc.vector._custom_dve(TENT, out=tent[:, c * NB:(c + 1) * NB],
                                          in0=src_fp32, s0=float(c) - 3.5, s1=2.0,
                                          imm2=2.5)
                    nc.vector._custom_dve(CUBE, out=planes[:, c * NB:(c + 1) * NB],
                                          in0=tent[:, c * NB:(c + 1) * NB], imm2=4.0)
                    rhs.append(planes[:, c * NB:(c + 1) * NB])
                return rhs

            def group(src_fp32, wd, kt, npl, with_sil=True, is_start=False):
                # j-inner: one feature block -> (with_sil + npl) k-tiles
                rhs_list = dve_rhs(src_fp32, npl, with_sil)
                for j, rhs in enumerate(rhs_list):
                    w = wtile(wd, kt + j)
                    for mt in range(FB):
                        nc.tensor.matmul(ps[mt][:], w[:, mt * 128:(mt + 1) * 128],
                                         rhs, start=(is_start and kt + j == 0),
                                         stop=False)
                return kt + len(rhs_list)

            def group_mt_outer(src_fp32, wd, kt, npl, epilogue, with_sil=True):
                # last group of a phase: mt-outer so ps[mt] finishes early and
                # the per-mt epilogue (act + store) overlaps remaining matmuls
                rhs_list = dve_rhs(src_fp32, npl, with_sil)
                ws = [wtile(wd, kt + j) for j in range(len(rhs_list))]
                last = len(rhs_list) - 1
                for mt in range(FB):
                    for j, rhs in enumerate(rhs_list):
                        nc.tensor.matmul(ps[mt][:], ws[j][:, mt * 128:(mt + 1) * 128],
                                         rhs, start=False, stop=(j == last))
                    epilogue(mt)
                return kt + len(rhs_list)

            def dma_only():
                for kt in range(KT_A):
                    w = wpool.tile([128, H], wdt, name="w")
                    nc.sync.dma_start(w[:], wa_d[kt * 128:(kt + 1) * 128, :])
                for kt in range(KT_B):
                    w = wpool.tile([128, H], wdt, name="w")
                    nc.sync.dma_start(w[:], wb_d[kt * 128:(kt + 1) * 128, :])

            def ep_a(mt):
                # tanh into oT; store o in 1024-wide chunks (fewer DGE configs)
                sl = slice(mt * NB, (mt + 1) * NB)
                nc.scalar.activation(oT[:, sl], ps[mt][:], AF.Tanh)
                if mt % 2 == 1:
                    sl2 = slice((mt - 1) * NB, (mt + 1) * NB)
                    nc.scalar.dma_start(o_d[:, sl2], oT[:, sl2])

            def ep_b(mt):
                sl = slice(mt * NB, (mt + 1) * NB)
                if mt == 7:
                    # final epilogue is the kernel tail: act in halves so the
                    # first store overlaps the second activation
                    for h in range(2):
                        sh = slice(mt * NB + h * (NB // 2),
                                   mt * NB + (h + 1) * (NB // 2))
                        nc.scalar.activation(zt[:, sh], ps[mt][:, h * (NB // 2):(h + 1) * (NB // 2)],
                                             AF.Identity, bias=hb[:, mt:mt + 1],
                                             scale=1.0)
                        nc.sync.dma_start(zo_d[:, sh], zt[:, sh])
                    return
                nc.scalar.activation(zt[:, sl], ps[mt][:], AF.Identity,
                                     bias=hb[:, mt:mt + 1], scale=1.0)
                # 1024-wide chunks early; the last one on the idle SP queue so
                # the DGE config overlaps the act and the tail latency shrinks
                if mt in (1, 3, 5):
                    sl2 = slice((mt - 1) * NB, (mt + 1) * NB)
                    nc.scalar.dma_start(zo_d[:, sl2], zt[:, sl2])
                elif mt == 6:
                    nc.sync.dma_start(zo_d[:, sl], zt[:, sl])

            for _rep in range(reps):
                if mode == "dma":
                    dma_only()
                    continue
                # phase A: s = kan_x(x) + kan_h(z); o = tanh(s)
                # Input chunks come from a 2-slot pool: chunk g+2's DMA gains
                # a WAR dep on chunk g's last reader, which paces the loads at
                # group rate in HARDWARE (no startup flood racing the weight
                # stream for HBM). Chunk 0 goes program-first on the SP queue
                # (ahead of the weight tiles on the same FIFO) and the Act
                # queue carries nothing but [table load, silus, epilogues], so
                # the first matmul can issue as soon as xs0+w0 land.
                # HAM warm-up: ~8 zero matmuls fill the otherwise-idle PE
                # window while xs0/w0 stream in, so the activity monitor
                # un-throttles the PE clock before the real stream begins.
                nc.gpsimd.memset(warm[:], 0.0)
                for _w in range(8):
                    nc.tensor.matmul(ps[0][:], warm[:, 0:128], warm[:],
                                     start=True, stop=True)

                kt = 0
                for src_d, is_z in ((xs_d, False), (zs_d, True)):
                    for fb in range(FB):
                        sl = slice(fb * NB, (fb + 1) * NB)
                        chunk = cpool.tile([128, NB], FP32, name="chunk")
                        nc.sync.dma_start(chunk[:], src_d[:, sl])
                        if kt == KT_A - 1 - NPL:
                            kt = group_mt_outer(chunk[:], wa_d, kt, NPL, ep_a)
                        else:
                            kt = group(chunk[:], wa_d, kt, NPL, is_start=True)
                        if is_z:
                            # cast on the idle Pool engine: keeps the Act
                            # engine on silu only (no act-table switches)
                            nc.gpsimd.tensor_copy(zb[:, sl], chunk[:])

                # phase B: z_out = z @ hh_w.T + hb + kan_hz(o)
                nc.gpsimd.dma_start(hb[:], hb_d[:])
                kt = 0
                for fb in range(FB):
                    w = wtile(wb_d, kt)
                    rhs = zb[:, fb * NB:(fb + 1) * NB]
                    for mt in range(FB):
                        nc.tensor.matmul(ps[mt][:], w[:, mt * 128:(mt + 1) * 128],
                                         rhs, start=(kt == 0), stop=False)
                    kt += 1
                for fb in range(FB):
                    sl = slice(fb * NB, (fb + 1) * NB)
                    if fb == FB - 1:
                        kt = group_mt_outer(oT[:, sl], wb_d, kt, NPL_B, ep_b,
                                            with_sil=False)
                    else:
                        kt = group(oT[:, sl], wb_d, kt, NPL_B, with_sil=False)

    bass_rust.generate_event_semaphores(nc)
    mybir.codegen_inst_isa_subclasses(nc)
    return nc


_NCS = {}


def _get_nc(reps=1, mode="full"):
    key = (reps, mode)
    if key not in _NCS:
        _NCS[key] = _build(reps, mode)
    return _NCS[key]


def _to_dev(a):  # [NB, H] -> [128, FB*NB], block fb holds features fb*128..+128
    return np.ascontiguousarray(
        a.reshape(NB, FB, 128).transpose(2, 1, 0).reshape(128, FB * NB))


def _from_dev(a):  # [128, FB*NB] -> [NB, H]
    return a.reshape(128, FB, NB).transpose(2, 1, 0).reshape(NB, H)


def _pack_kan(wa, kt, bw, sw, sc):
    w2 = (np.asarray(sw, np.float32) * np.asarray(sc, np.float32)[:, :, None]) / 6.0
    bw = np.asarray(bw, np.float32)
    for fb in range(FB):
        wa[kt * 128:(kt + 1) * 128] = bw[:, fb * 128:(fb + 1) * 128].T
        kt += 1
        for c in range(NPL):
            wa[kt * 128:(kt + 1) * 128] = w2[:, fb * 128:(fb + 1) * 128, c].T
            kt += 1
    return kt


def _pack_kan_bounded(wb, kt, bw, sw, sc):
    # input bounded in (-1,1): silu folds into the spline basis (BETA_SILU)
    # and partition of unity folds plane 7 into a bias. Returns (kt, bias[o]).
    sw = np.asarray(sw, np.float32)
    sc = np.asarray(sc, np.float32)
    bw = np.asarray(bw, np.float32)
    wc = sw * sc[:, :, None] + bw[:, :, None] * BETA_SILU[None, None, :]
    bias = wc[:, :, NPL - 1].sum(axis=1)
    w2 = (wc[:, :, :NPL_B] - wc[:, :, NPL - 1:NPL]) / 6.0
    for fb in range(FB):
        for c in range(NPL_B):
            wb[kt * 128:(kt + 1) * 128] = w2[:, fb * 128:(fb + 1) * 128, c].T
            kt += 1
    return kt, bias


def _make_in_maps(inputs):
    x = np.ascontiguousarray(np.asarray(inputs["x_t"], np.float32))
    z = np.ascontiguousarray(np.asarray(inputs["z_prev"], np.float32))

    wa = np.empty((KT_A * 128, H), np.float32)
    kt = _pack_kan(wa, 0, inputs["wx_base"], inputs["wx_spline"], inputs["wx_scaler"])
    kt = _pack_kan(wa, kt, inputs["wh_base"], inputs["wh_spline"], inputs["wh_scaler"])
    assert kt == KT_A

    wb = np.empty((KT_B * 128, H), np.float32)
    wb[:H] = np.asarray(inputs["hh_w"], np.float32).T
    kt, hz_bias = _pack_kan_bounded(wb, FB, inputs["hz_base"], inputs["hz_spline"],
                                    inputs["hz_scaler"])
    assert kt == KT_B
    wa = np.ascontiguousarray(wa.astype(BF16_NP))
    wb = np.ascontiguousarray(wb.astype(BF16_NP))

    hbias = np.asarray(inputs["hh_b"], np.float32) + hz_bias
    hb = np.ascontiguousarray(hbias.reshape(FB, 128).T)

    return [{"xs": _to_dev(x[d * NB:(d + 1) * NB]),
             "zs": _to_dev(z[d * NB:(d + 1) * NB]),
             "wa": wa, "wb": wb, "hb": hb} for d in range(NCORES)]


def _run(inputs, trace=False):
    nc = _get_nc()
    in_maps = _make_in_maps(inputs)
    res = run_bass_kernel_spmd(nc, in_maps, list(range(NCORES)), trace=trace)
    o = np.empty((B, H), np.float32)
    zt = np.empty((B, H), np.float32)
    for d in range(NCORES):
        o[d * NB:(d + 1) * NB] = _from_dev(res.results[d]["o"])
        zt[d * NB:(d + 1) * NB] = _from_dev(
            np.asarray(res.results[d]["zo"]).astype(np.float32))
    return (o, zt), res


def kernel(**inputs):
    return _run(inputs, trace=False)[0]



# revision 8
# speedup vs baseline: 1.2040x; 1.0001x over previous
import sys

sys.path.insert(0, "/opt/trn_rl_repo")

import numpy as np
import bass_rust
import concourse.bass as bass
import concourse.mybir as mybir
import concourse.tile as tile
from concourse.bass_utils import run_bass_kernel_spmd

import concourse.dve_ops as dve_ops
from concourse.dve_ops import DveOp
from concourse.dve_spec import Spec, Src0, C0, C1, C2, One, maxx, relu, sq, lower, _has_src1
from concourse.dve_uop import DveOpSpec

B, H, NCORES = 4096, 1024, 8
NB = B // NCORES          # 512 batch rows per core
FB = H // 128             # 8 feature blocks of 128
NPL = 8                   # spline planes per feature (phase A)
NPL_B = 7                 # phase B: |o|<1 => partition of unity folds plane 7
KT_A = 2 * FB * (1 + NPL)  # 144 k-tiles: kan_x + kan_h
KT_B = FB + FB * NPL_B     # 64 k-tiles: hh_w + kan_hz (silu folded into planes)
FP32 = mybir.dt.float32
BF16 = mybir.dt.bfloat16
BF16_NP = mybir.dt.np(BF16)
AF = mybir.ActivationFunctionType


def _register_dve(name, spec):
    if name in dve_ops._SUB_OPCODE_FOR_NAME:
        return next(op for op in dve_ops.OPS if op.name == name)
    row = dve_ops._CUSTOM_DVE_ROW_BASE + len(dve_ops.OPS)
    assert row < 0x20
    dve_ops._SUB_OPCODE_FOR_NAME[name] = row
    shas = {v: DveOpSpec(name=name, opcode=row, uops=lower(spec, ver=v),
                         rd1_en=_has_src1(spec)).sha(v) for v in ("v3", "v4")}
    op = DveOp(name, spec, subdim=False, uops_sha=shas)
    dve_ops.OPS.append(op)
    dve_ops.CUSTOM_DVE_SPECS[name] = op.spec
    return op


# p_c = relu(2 - |2.5*x - (c - 3.5)|), then plane = p^3 - 4*relu(p-1)^3 = 6*B3(u-c)
_m = Src0 * C2
TENT = _register_dve("KAN_TENT", Spec(
    body=relu(C1 - maxx(_m - C0, C0 - _m)),
    reference=lambda in0, in1, s0, s1, imm2:
        np.maximum(s1 - np.abs(in0.astype(np.float32) * imm2 - s0), 0.0)))
_q = relu(Src0 - One)
CUBE = _register_dve("KAN_CUBE", Spec(
    body=sq(Src0) * Src0 - sq(_q) * _q * C2,
    reference=lambda in0, in1, s0, s1, imm2:
        in0.astype(np.float32) ** 3
        - imm2 * np.maximum(in0.astype(np.float32) - 1.0, 0.0) ** 3))


def _beta_silu():
    # coefficients of silu in the 8-fn cubic B-spline basis on [-1,1]
    # (max residual ~1.9e-5); lets phase B drop its silu k-tiles entirely
    x = np.linspace(-1.0, 1.0, 4001)
    c = np.arange(NPL)[None, :]
    p = np.maximum(2.0 - np.abs(2.5 * x[:, None] - (c - 3.5)), 0.0)
    A = (p ** 3 - 4.0 * np.maximum(p - 1.0, 0.0) ** 3) / 6.0
    silu = x / (1.0 + np.exp(-x))
    return np.linalg.lstsq(A, silu, rcond=None)[0].astype(np.float32)


BETA_SILU = _beta_silu()


def _build(reps=1, mode="full"):
    # mode: "full" | "pe" (no weight streaming, one resident w tile)
    #       | "dma" (weight streaming only, no compute)
    wdt = BF16
    nc = bass.Bass(target_bir_lowering=False)
    xs_d = nc.dram_tensor("xs", [128, FB * NB], FP32, kind="ExternalInput")
    zs_d = nc.dram_tensor("zs", [128, FB * NB], FP32, kind="ExternalInput")
    wa_d = nc.dram_tensor("wa", [KT_A * 128, H], wdt, kind="ExternalInput")
    wb_d = nc.dram_tensor("wb", [KT_B * 128, H], wdt, kind="ExternalInput")
    hb_d = nc.dram_tensor("hb", [128, FB], FP32, kind="ExternalInput")
    o_d = nc.dram_tensor("o", [128, FB * NB], FP32, kind="ExternalOutput")
    zo_d = nc.dram_tensor("zo", [128, FB * NB], BF16, kind="ExternalOutput")

    with tile.TileContext(nc) as tc:
        with tc.tile_pool(name="sbuf", bufs=1) as pool, \
             tc.tile_pool(name="wp", bufs=16) as wpool, \
             tc.tile_pool(name="cp", bufs=3) as cpool, \
             tc.tile_pool(name="dp", bufs=2) as dpool, \
             tc.tile_pool(name="psum", bufs=1, space="PSUM") as psum:
            zb = pool.tile([128, FB * NB], BF16)   # bf16 copy of z for hh matmul
            oT = pool.tile([128, FB * NB], FP32)
            zt = pool.tile([128, FB * NB], BF16)
            hb = pool.tile([128, FB], FP32)
            warm = pool.tile([128, NB], BF16)      # zeros; HAM warm-up operand

            ps = [psum.tile([128, NB], FP32, name=f"ps{mt}") for mt in range(FB)]

            w_res = None
            if mode == "pe":
                w_res = pool.tile([128, H], wdt)
                nc.sync.dma_start(w_res[:], wa_d[0:128, :])

            def wtile(wd, kt):
                if mode == "pe":
                    return w_res
                w = wpool.tile([128, H], wdt, name="w")
                nc.sync.dma_start(w[:], wd[kt * 128:(kt + 1) * 128, :])
                return w

            def dve_rhs(src_fp32, npl, with_sil):
                # [silu +] npl spline planes; per-plane TENT->CUBE so plane c
                # is ready early (the PE consumes plane c only at k-tile c+1)
                rhs = []
                if with_sil:
                    sil = dpool.tile([128, NB], BF16, name="sil")
                    nc.scalar.activation(sil[:], src_fp32, AF.Silu)
                    rhs.append(sil[:])
                tent = dpool.tile([128, NPL * NB], FP32, name="tent")
                planes = dpool.tile([128, NPL * NB], BF16, name="planes")
                for c in range(npl):
                    nc.vector._custom_dve(TENT, out=tent[:, c * NB:(c + 1) * NB],
                                          in0=src_fp32, s0=float(c) - 3.5, s1=2.0,
                                          imm2=2.5)
                    nc.vector._custom_dve(CUBE, out=planes[:, c * NB:(c + 1) * NB],
                                          in0=tent[:, c * NB:(c + 1) * NB], imm2=4.0)
                    rhs.append(planes[:, c * NB:(c + 1) * NB])
                return rhs

            def group(src_fp32, wd, kt, npl, with_sil=True, is_start=False):
                # j-inner: one feature block -> (with_sil + npl) k-tiles
                rhs_list = dve_rhs(src_fp32, npl, with_sil)
                for j, rhs in enumerate(rhs_list):
                    w = wtile(wd, kt + j)
                    for mt in range(FB):
                        nc.tensor.matmul(ps[mt][:], w[:, mt * 128:(mt + 1) * 128],
                                         rhs, start=(is_start and kt + j == 0),
                                         stop=False)
                return kt + len(rhs_list)

            def group_mt_outer(src_fp32, wd, kt, npl, epilogue, with_sil=True):
                # last group of a phase: mt-outer so ps[mt] finishes early and
                # the per-mt epilogue (act + store) overlaps remaining matmuls
                rhs_list = dve_rhs(src_fp32, npl, with_sil)
                ws = [wtile(wd, kt + j) for j in range(len(rhs_list))]
                last = len(rhs_list) - 1
                for mt in range(FB):
                    for j, rhs in enumerate(rhs_list):
                        nc.tensor.matmul(ps[mt][:], ws[j][:, mt * 128:(mt + 1) * 128],
                                         rhs, start=False, stop=(j == last))
                    epilogue(mt)
                return kt + len(rhs_list)

            def dma_only():
                for kt in range(KT_A):
                    w = wpool.tile([128, H], wdt, name="w")
                    nc.sync.dma_start(w[:], wa_d[kt * 128:(kt + 1) * 128, :])
                for kt in range(KT_B):
                    w = wpool.tile([128, H], wdt, name="w")
                    nc.sync.dma_start(w[:], wb_d[kt * 128:(kt + 1) * 128, :])

            def ep_a(mt):
                # tanh into oT; store o in 1024-wide chunks (fewer DGE configs)
                sl = slice(mt * NB, (mt + 1) * NB)
                nc.scalar.activation(oT[:, sl], ps[mt][:], AF.Tanh)
                if mt % 2 == 1:
                    sl2 = slice((mt - 1) * NB, (mt + 1) * NB)
                    nc.scalar.dma_start(o_d[:, sl2], oT[:, sl2])

            def ep_b(mt):
                sl = slice(mt * NB, (mt + 1) * NB)
                if mt == 7:
                    # final epilogue is the kernel tail: act in halves so the
                    # first store overlaps the second activation
                    for h in range(2):
                        sh = slice(mt * NB + h * (NB // 2),
                                   mt * NB + (h + 1) * (NB // 2))
                        nc.scalar.activation(zt[:, sh], ps[mt][:, h * (NB // 2):(h + 1) * (NB // 2)],
                                             AF.Identity, bias=hb[:, mt:mt + 1],
                                             scale=1.0)
                        nc.sync.dma_start(zo_d[:, sh], zt[:, sh])
                    return
                nc.scalar.activation(zt[:, sl], ps[mt][:], AF.Identity,
                                     bias=hb[:, mt:mt + 1], scale=1.0)
                # 1024-wide chunks early; the last one on the idle SP queue so
                # the DGE config overlaps the act and the tail latency shrinks
                if mt in (1, 3, 5):
                    sl2 = slice((mt - 1) * NB, (mt + 1) * NB)
                    nc.scalar.dma_start(zo_d[:, sl2], zt[:, sl2])
                elif mt == 6:
                    nc.sync.dma_start(zo_d[:, sl], zt[:, sl])

            for _rep in range(reps):
                if mode == "dma":
                    dma_only()
                    continue
                # phase A: s = kan_x(x) + kan_h(z); o = tanh(s)
                # Input chunks come from a 3-slot pool: chunk g+3's DMA gains
                # a WAR dep on chunk g's last reader, which paces the loads at
                # group rate in HARDWARE (no startup flood racing the weight
                # stream for HBM), and all ride the SP queue program-ordered
                # with the weight tiles (single FIFO, no cross-queue races).
                # The Act queue carries nothing but [table load, silus,
                # epilogues], so the first matmul issues as soon as xs0+w0 land.
                # HAM warm-up: ~8 zero matmuls fill the otherwise-idle PE
                # window while xs0/w0 stream in, so the activity monitor
                # un-throttles the PE clock before the real stream begins.
                nc.gpsimd.memset(warm[:], 0.0)
                for _w in range(8):
                    nc.tensor.matmul(ps[0][:], warm[:, 0:128], warm[:],
                                     start=True, stop=True)

                kt = 0
                for src_d, is_z in ((xs_d, False), (zs_d, True)):
                    for fb in range(FB):
                        sl = slice(fb * NB, (fb + 1) * NB)
                        chunk = cpool.tile([128, NB], FP32, name="chunk")
                        nc.sync.dma_start(chunk[:], src_d[:, sl])
                        if kt == KT_A - 1 - NPL:
                            kt = group_mt_outer(chunk[:], wa_d, kt, NPL, ep_a)
                        else:
                            kt = group(chunk[:], wa_d, kt, NPL, is_start=True)
                        if is_z:
                            # cast on the idle Pool engine: keeps the Act
                            # engine on silu only (no act-table switches)
                            nc.gpsimd.tensor_copy(zb[:, sl], chunk[:])

                # phase B: z_out = z @ hh_w.T + hb + kan_hz(o)
                nc.gpsimd.dma_start(hb[:], hb_d[:])
                kt = 0
                for fb in range(FB):
                    w = wtile(wb_d, kt)
                    rhs = zb[:, fb * NB:(fb + 1) * NB]
                    for mt in range(FB):
                        nc.tensor.matmul(ps[mt][:], w[:, mt * 128:(mt + 1) * 128],
                                         rhs, start=(kt == 0), stop=False)
                    kt += 1
                for fb in range(FB):
                    sl = slice(fb * NB, (fb + 1) * NB)
                    if fb == FB - 1:
                        kt = group_mt_outer(oT[:, sl], wb_d, kt, NPL_B, ep_b,
                                            with_sil=False)
                    else:
                        kt = group(oT[:, sl], wb_d, kt, NPL_B, with_sil=False)

    bass_rust.generate_event_semaphores(nc)
    mybir.codegen_inst_isa_subclasses(nc)
    return nc


_NCS = {}


def _get_nc(reps=1, mode="full"):
    key = (reps, mode)
    if key not in _NCS:
        _NCS[key] = _build(reps, mode)
    return _NCS[key]


def _to_dev(a):  # [NB, H] -> [128, FB*NB], block fb holds features fb*128..+128
    return np.ascontiguousarray(
        a.reshape(NB, FB, 128).transpose(2, 1, 0).reshape(128, FB * NB))


def _from_dev(a):  # [128, FB*NB] -> [NB, H]
    return a.reshape(128, FB, NB).transpose(2, 1, 0).reshape(NB, H)


def _pack_kan(wa, kt, bw, sw, sc):
    w2 = (np.asarray(sw, np.float32) * np.asarray(sc, np.float32)[:, :, None]) / 6.0
    bw = np.asarray(bw, np.float32)
    for fb in range(FB):
        wa[kt * 128:(kt + 1) * 128] = bw[:, fb * 128:(fb + 1) * 128].T
        kt += 1
        for c in range(NPL):
            wa[kt * 128:(kt + 1) * 128] = w2[:, fb * 128:(fb + 1) * 128, c].T
            kt += 1
    return kt


def _pack_kan_bounded(wb, kt, bw, sw, sc):
    # input bounded in (-1,1): silu folds into the spline basis (BETA_SILU)
    # and partition of unity folds plane 7 into a bias. Returns (kt, bias[o]).
    sw = np.asarray(sw, np.float32)
    sc = np.asarray(sc, np.float32)
    bw = np.asarray(bw, np.float32)
    wc = sw * sc[:, :, None] + bw[:, :, None] * BETA_SILU[None, None, :]
    bias = wc[:, :, NPL - 1].sum(axis=1)
    w2 = (wc[:, :, :NPL_B] - wc[:, :, NPL - 1:NPL]) / 6.0
    for fb in range(FB):
        for c in range(NPL_B):
            wb[kt * 128:(kt + 1) * 128] = w2[:, fb * 128:(fb + 1) * 128, c].T
            kt += 1
    return kt, bias


def _make_in_maps(inputs):
    x = np.ascontiguousarray(np.asarray(inputs["x_t"], np.float32))
    z = np.ascontiguousarray(np.asarray(inputs["z_prev"], np.float32))

    wa = np.empty((KT_A * 128, H), np.float32)
    kt = _pack_kan(wa, 0, inputs["wx_base"], inputs["wx_spline"], inputs["wx_scaler"])
    kt = _pack_kan(wa, kt, inputs["wh_base"], inputs["wh_spline"], inputs["wh_scaler"])
    assert kt == KT_A

    wb = np.empty((KT_B * 128, H), np.float32)
    wb[:H] = np.asarray(inputs["hh_w"], np.float32).T
    kt, hz_bias = _pack_kan_bounded(wb, FB, inputs["hz_base"], inputs["hz_spline"],
                                    inputs["hz_scaler"])
    assert kt == KT_B
    wa = np.ascontiguousarray(wa.astype(BF16_NP))
    wb = np.ascontiguousarray(wb.astype(BF16_NP))

    hbias = np.asarray(inputs["hh_b"], np.float32) + hz_bias
    hb = np.ascontiguousarray(hbias.reshape(FB, 128).T)

    return [{"xs": _to_dev(x[d * NB:(d + 1) * NB]),
             "zs": _to_dev(z[d * NB:(d + 1) * NB]),
             "wa": wa, "wb": wb, "hb": hb} for d in range(NCORES)]


def _run(inputs, trace=False):
    nc = _get_nc()
    in_maps = _make_in_maps(inputs)
    res = run_bass_kernel_spmd(nc, in_maps, list(range(NCORES)), trace=trace)
    o = np.empty((B, H), np.float32)
    zt = np.empty((B, H), np.float32)
    for d in range(NCORES):
        o[d * NB:(d + 1) * NB] = _from_dev(res.results[d]["o"])
        zt[d * NB:(d + 1) * NB] = _from_dev(
            np.asarray(res.results[d]["zo"]).astype(np.float32))
    return (o, zt), res


def kernel(**inputs):
    return _run(inputs, trace=False)[0]

